# revision 17
# baseline (speedup 1.0000x reference)
"""Trainium2 Bass kernel for nn_Attention_8323646620215.

LayerNorm -> QKV -> scores(+rel-bias+mask) -> softmax -> attn@V -> out proj.

Sharding: 8 cores = (batch b in 0..3) x (query-half in 0..1). Each core
computes the full K/V for its batch and attention for its 1024 query rows;
no cross-core communication.

v2 design: one ACT-exp-bound pipeline. The softmax exp (128 x [128,1024]
f32 PSUM reads on the scalar engine) is the hard floor (~128us); everything
else is scheduled into the other engines' slack under it:

  - No on-chip transposes: the host also sends xT = x.T (bf16). Q/K/V are
    computed from RAW xT (un-normalized); LayerNorm enters algebraically:
      xn = (x - mu) * rstd  (gamma/beta folded into weights host-side)
      K_hat[out, tok] = rstd[tok] * (W.T @ xT - g (x) mu)[out, tok]
    The rank-1 mean term (g = colsum(W)) is one extra contraction-1 matmul
    accumulated into the same PSUM group; the per-token rstd scale is
    applied at PSUM evacuation (DVE tensor_tensor with a broadcast rstd_T
    tile built on-chip by a rank-1 PE matmul).
  - mu/rstd come from DVE bn_stats on the f32 x, shipped through a tiny
    DRAM roundtrip (gpsimd cast-DMA back as bf16 rows).
  - expA = exp(clip rel bias) * mask is precomputed host-side ([N, NQ]
    bf16, same bytes as the old mask DMA) - no Toeplitz build on-chip.
  - Phase C runs 8 passes (head-pair x query-half) x 16 key tiles:
    scores (2 row-packed 64-contraction matmuls) -> exp -> pb = eb*expA
    (stride-0 repeated read) -> attn@V (65-row stationary, ones row gives
    the softmax denominator). AV matmuls run LAG iterations behind the exp
    stream so V/K/Q production for later passes interleaves into PE slack.
  - Denominators: DRAM roundtrip reshape -> reciprocal -> broadcast read
    (as in v1), interleaved into the following pass.
"""
import sys
import types
import numpy as np

sys.path.insert(0, "/opt/trn_rl_repo")

# ---- environment fixes (axon agent container) -------------------------------
if "antenv.axon_hooks" not in sys.modules:
    _m = types.ModuleType("antenv.axon_hooks")
    _m._hook = None
    _m.set_axon_ntff_profile_hook = lambda h: setattr(_m, "_hook", h)
    _m.get_axon_ntff_profile_hook = lambda: _m._hook
    sys.modules["antenv.axon_hooks"] = _m
    try:
        from trn_agent_boot.trn_boot import _ntff_profile_via_ctypes
        _m._hook = _ntff_profile_via_ctypes("/opt/axon/libaxon_pjrt.so")
    except Exception:
        pass

import ml_dtypes  # noqa: E402
from concourse import bass, mybir, tile  # noqa: E402
from concourse.bass_utils import run_bass_kernel_spmd  # noqa: E402

F32 = mybir.dt.float32
BF16 = mybir.dt.bfloat16
AF = mybir.ActivationFunctionType
OP = mybir.AluOpType

B, N, D, H, DH, MAXREL = 4, 2048, 512, 8, 64, 200
NQ = N // 2          # queries per core
NT = N // 128        # 16 token tiles
NCORES = 8
LAG = 5              # AV matmuls trail the exp stream by this many iters

# This container's walrus rejects instructions with more than one sem wait.
# Splitting is sound: a same-engine NoOp right before the instruction
# enforces the wait at the same program point (sequencers run in order).


def _split_waits(nc, maxw=1):
    n_split = 0
    for f in nc.m.functions:
        for blk in f.blocks:
            bb = blk.bb if hasattr(blk, "bb") else blk
            insts = list(bb.instructions)
            out = []
            changed = False
            for inst in insts:
                si = inst.sync_info
                waits = list(si.on_wait) if si and si.on_wait else []
                if len(waits) > maxw:
                    extra = waits[:-maxw]
                    chunks = [extra[j:j + maxw] for j in range(0, len(extra), maxw)]
                    for i, chunk in enumerate(chunks):
                        nop = mybir.InstNoOp(name=f"{inst.name}-ws{i}", ins=[], outs=[])
                        nop.engine = inst.engine
                        nop.sync_info = mybir.SyncInfo(on_wait=chunk, on_update=[])
                        out.append(nop)
                    si.on_wait = waits[-maxw:]
                    changed = True
                    n_split += 1
                out.append(inst)
            if changed:
                bb.instructions = out
    return n_split


def build(has_c=False, has_b=False, split_waits=True):
    nc = bass.Bass("TRN2", target_bir_lowering=False, debug=False,
                   num_devices=NCORES)
    x_d = nc.dram_tensor("x", [N, D], F32, kind="ExternalInput")
    xt_d = nc.dram_tensor("xt", [D, N], BF16, kind="ExternalInput")
    wqkv_d = nc.dram_tensor("wqkv", [D, 3 * D], BF16, kind="ExternalInput")
    grow_d = nc.dram_tensor("grow", [1, 3 * D], BF16, kind="ExternalInput")
    wout_d = nc.dram_tensor("wout", [D, D], BF16, kind="ExternalInput")
    bout_d = nc.dram_tensor("bout", [D], F32, kind="ExternalInput")
    cqkv_d = nc.dram_tensor("cqkv", [3 * D], F32, kind="ExternalInput")
    expa_d = nc.dram_tensor("expa", [N, NQ], BF16, kind="ExternalInput")
    mu_d = nc.dram_tensor("mu_scratch", [1, N], F32)
    rs_d = nc.dram_tensor("rs_scratch", [1, N], F32)
    dsb_d = nc.dram_tensor("den_scratch", [H, NQ], BF16)
    dsi_d = nc.dram_tensor("invden_scratch", [H, NQ], BF16)
    y_d = nc.dram_tensor("y", [NQ, D], F32, kind="ExternalOutput")

    with tile.TileContext(nc) as tc, \
         tc.tile_pool(name="const", bufs=1) as C, \
         tc.tile_pool(name="pers", bufs=1) as P, \
         tc.tile_pool(name="work", bufs=3) as W:

        # ---- persistent tiles ----------------------------------------------
        xt_sb = [P.tile([128, N], BF16, tag=f"xt{fb}", name=f"xt{fb}")
                 for fb in range(4)]
        KTp = [P.tile([128, N], BF16, tag=f"KT{hp}", name=f"KT{hp}") for hp in range(4)]
        QTp = [P.tile([128, NQ], BF16, tag=f"QT{hp}", name=f"QT{hp}") for hp in range(4)]
        Vau = [P.tile([128, H, 66], BF16, tag=f"V{t}", name=f"Vau{t}") for t in range(NT)]
        expA = [P.tile([128, NQ], BF16, tag=f"eA{t}", name=f"eA{t}") for t in range(NT)]
        numT = [P.tile([65, NQ], BF16, tag=f"nT{h}", name=f"nT{h}") for h in range(H)]
        pairT = [P.tile([128, NQ], BF16, tag=f"pT{hp}", name=f"pT{hp}") for hp in range(4)]
        rstdT = P.tile([128, N], BF16, tag="rstdT", name="rstdT")
        mu_all = P.tile([128, NT], F32, tag="mu_all")
        rs_all = P.tile([128, NT], F32, tag="rs_all")
        murow = P.tile([1, N], BF16, tag="murow")
        rsrow = P.tile([1, N], BF16, tag="rsrow")

        # ---- DMA issue (per-queue program order = transfer order) ----------
        # scalar HWDGE queue: wqkv, xT (chunk-major), early expA, mu/rs outs
        wqkv_sb = [C.tile([128, 3 * D], BF16, tag=f"wqkv{kb}", name=f"wqkv{kb}")
                   for kb in range(4)]
        for kb in range(4):
            nc.scalar.dma_start(out=wqkv_sb[kb][:],
                                in_=wqkv_d[kb * 128:(kb + 1) * 128, :])
        for c in range(4):
            for fb in range(4):
                nc.scalar.dma_start(
                    out=xt_sb[fb][:, c * 512:(c + 1) * 512],
                    in_=xt_d[fb * 128:(fb + 1) * 128, c * 512:(c + 1) * 512])
            for t in (2 * c, 2 * c + 1):
                nc.scalar.dma_start(out=expA[t][:],
                                    in_=expa_d[t * 128:(t + 1) * 128, :])
        # sync HWDGE queue: x tiles, late expA tiles
        x_ts = [W.tile([128, D], F32, tag="x", bufs=6, name=f"x{t}")
                for t in range(NT)]
        for t in range(NT):
            nc.sync.dma_start(out=x_ts[t][:], in_=x_d[t * 128:(t + 1) * 128, :])
        for t in range(8, NT):
            nc.sync.dma_start(out=expA[t][:],
                              in_=expa_d[t * 128:(t + 1) * 128, :])
        # gpsimd SWDGE queue: wout, grow, optional bias tiles
        woutP = [C.tile([128, D], BF16, tag=f"woutP{hp}", name=f"woutP{hp}")
                 for hp in range(4)]
        for hp in range(4):
            nc.gpsimd.dma_start(out=woutP[hp][:],
                                in_=wout_d[hp * 128:(hp + 1) * 128, :])
        grow_sb = C.tile([1, 3 * D], BF16, tag="grow")
        nc.gpsimd.dma_start(out=grow_sb[:], in_=grow_d[0:1, :])
        if has_c:
            cq_all = C.tile([128, 12], F32, tag="cq")
            nc.gpsimd.dma_start(
                out=cq_all[:],
                in_=bass.AP(tensor=cqkv_d.ap().tensor, offset=0,
                            ap=[[1, 128], [128, 12]]))
            cv_bc = C.tile([128, D], F32, tag="cv")
            nc.gpsimd.dma_start(
                out=cv_bc[:],
                in_=bass.AP(tensor=cqkv_d.ap().tensor, offset=2 * D,
                            ap=[[0, 128], [1, D]]))
        if has_b:
            bout_bc = C.tile([128, D], F32, tag="bout")
            nc.gpsimd.dma_start(
                out=bout_bc[:],
                in_=bass.AP(tensor=bout_d.ap().tensor, offset=0,
                            ap=[[0, 128], [1, D]]))

        ones1 = C.tile([1, 128], BF16, tag="ones1")
        nc.vector.memset(ones1[:], 1.0)
        eps_t = C.tile([128, 1], F32, tag="eps")
        nc.vector.memset(eps_t[:], 1e-5)

        # mu/rs roundtrip per 4-tile chunk: out on scalar queue (f32), back
        # on gpsimd (cast to bf16 rows). Must be EMITTED after the stats
        # that write mu_all/rs_all (program order is logical order in Tile).
        def roundtrip(c):
            sl_s = [[1, 128], [128, 4]]
            nc.scalar.dma_start(
                out=bass.AP(tensor=mu_d.ap().tensor, offset=512 * c, ap=sl_s),
                in_=mu_all[:, 4 * c:4 * c + 4])
            nc.scalar.dma_start(
                out=bass.AP(tensor=rs_d.ap().tensor, offset=512 * c, ap=sl_s),
                in_=rs_all[:, 4 * c:4 * c + 4])
            nc.gpsimd.dma_start(out=murow[0:1, 512 * c:512 * (c + 1)],
                                in_=mu_d[0:1, 512 * c:512 * (c + 1)])
            nc.gpsimd.dma_start(out=rsrow[0:1, 512 * c:512 * (c + 1)],
                                in_=rs_d[0:1, 512 * c:512 * (c + 1)])

        # ---- LayerNorm stats (DVE+ACT only; no transposes, no apply) -------
        def ln_stats(t):
            st = W.tile([128, 6], F32, tag="st")
            nc.vector.bn_stats(out=st[:], in_=x_ts[t][:])
            mv = W.tile([128, 2], F32, tag="mv", bufs=4, name=f"mv{t}")
            nc.vector.bn_aggr(out=mv[:], in_=st[:])
            nc.vector.tensor_copy(out=mu_all[:, t:t + 1], in_=mv[:, 0:1])
            rsq = W.tile([128, 1], F32, tag="rsq", bufs=4, name=f"rsq{t}")
            nc.scalar.activation(out=rsq[:], in_=mv[:, 1:2], func=AF.Sqrt,
                                 bias=eps_t[:])
            nc.vector.reciprocal(out=rs_all[:, t:t + 1], in_=rsq[:])

        # ---- B-work closures (PE production of rstdT / K / Q / V) ----------
        with tc.tile_pool(name="psB", bufs=1, space="PSUM") as psB, \
             tc.tile_pool(name="psC", bufs=1, space="PSUM") as psC:

            def rstdT_build(c):
                def f():
                    bp = psB.tile([128, 512], F32, tag="bps", bufs=2)
                    nc.tensor.matmul(bp[:], ones1[:],
                                     rsrow[0:1, c * 512:(c + 1) * 512],
                                     start=True, stop=True)
                    nc.vector.tensor_copy(
                        out=rstdT[:, c * 512:(c + 1) * 512], in_=bp[:])
                return [f]

            def k_chunk(hp, c):
                """KTp[hp][:, c*512:(c+1)*512] (both heads' 128 rows)."""
                def mms():
                    kp = psB.tile([128, 512], F32, tag="bps", bufs=2)
                    for kb in range(4):
                        nc.tensor.matmul(
                            kp[:],
                            wqkv_sb[kb][:, D + hp * 128:D + (hp + 1) * 128],
                            xt_sb[kb][:, c * 512:(c + 1) * 512],
                            start=(kb == 0), stop=False)
                    nc.tensor.matmul(
                        kp[:], grow_sb[0:1, D + hp * 128:D + (hp + 1) * 128],
                        murow[0:1, c * 512:(c + 1) * 512],
                        start=False, stop=True)
                    sl = slice(c * 512, (c + 1) * 512)
                    nc.vector.tensor_mul(out=KTp[hp][:, sl], in0=kp[:],
                                         in1=rstdT[:, sl])
                    if has_c:
                        nc.vector.tensor_scalar_add(
                            out=KTp[hp][:, sl], in0=KTp[hp][:, sl],
                            scalar1=cq_all[:, 4 + hp:5 + hp])
                return [mms]

            def q_chunk(hp, ic):
                """QTp[hp][:, ic*512:(ic+1)*512] (queries = tokens ic-half)."""
                def mms():
                    qp = psB.tile([128, 512], F32, tag="bps", bufs=2)
                    for kb in range(4):
                        nc.tensor.matmul(
                            qp[:],
                            wqkv_sb[kb][:, hp * 128:(hp + 1) * 128],
                            xt_sb[kb][:, ic * 512:(ic + 1) * 512],
                            start=(kb == 0), stop=False)
                    nc.tensor.matmul(
                        qp[:], grow_sb[0:1, hp * 128:(hp + 1) * 128],
                        murow[0:1, ic * 512:(ic + 1) * 512],
                        start=False, stop=True)
                    sl = slice(ic * 512, (ic + 1) * 512)
                    nc.vector.tensor_mul(out=QTp[hp][:, sl], in0=qp[:],
                                         in1=rstdT[:, sl])
                    if has_c:
                        nc.vector.tensor_scalar_add(
                            out=QTp[hp][:, sl], in0=QTp[hp][:, sl],
                            scalar1=cq_all[:, hp:hp + 1])
                return [mms]

            def v_tile(t):
                """Vau[t][:, :, 0:64] = rstd * (x @ Wv - mu (x) g_v); ones col."""
                def mms():
                    vp = psB.tile([128, 512], F32, tag="bps", bufs=2)
                    for kb in range(4):
                        nc.tensor.matmul(
                            vp[:],
                            xt_sb[kb][:, t * 128:(t + 1) * 128],
                            wqkv_sb[kb][:, 2 * D:3 * D],
                            start=(kb == 0), stop=False)
                    nc.tensor.matmul(
                        vp[:], murow[0:1, t * 128:(t + 1) * 128],
                        grow_sb[0:1, 2 * D:3 * D],
                        start=False, stop=True)
                    nc.vector.memset(Vau[t][:, :, 64:65], 1.0)
                    nc.vector.tensor_scalar_mul(
                        out=Vau[t][:, :, 0:64], in0=vp[:],
                        scalar1=rs_all[:, t:t + 1])
                    if has_c:
                        nc.vector.tensor_add(out=Vau[t][:, :, 0:64],
                                             in0=Vau[t][:, :, 0:64], in1=cv_bc[:])
                return [mms]

            # ---- denominator pipeline (unchanged from v1) ------------------
            def den_pieces(hp):
                dal = W.tile([128, 2 * NQ // 128], BF16, tag="dall", bufs=2,
                             name=f"dal{hp}")
                dbs = [None, None]

                def p0():
                    nc.sync.dma_start(
                        out=dal[:],
                        in_=bass.AP(tensor=dsb_d.ap().tensor, offset=2 * hp * NQ,
                                    ap=[[2 * NQ // 128, 128], [1, 2 * NQ // 128]]))

                def p1():
                    nc.vector.tensor_scalar_add(out=dal[:], in0=dal[:],
                                                scalar1=1e-20)
                    with nc.allow_low_precision(reason="bf16 softmax denominators"):
                        nc.vector.reciprocal(out=dal[:], in_=dal[:])
                    nc.sync.dma_start(
                        out=bass.AP(tensor=dsi_d.ap().tensor, offset=2 * hp * NQ,
                                    ap=[[2 * NQ // 128, 128], [1, 2 * NQ // 128]]),
                        in_=dal[:])

                def load_bc(e):
                    def f():
                        h = 2 * hp + e
                        den_bc = W.tile([64, NQ], BF16, tag="denb", bufs=2,
                                        name=f"denb{h}")
                        dbs[e] = den_bc
                        nc.sync.dma_start(
                            out=den_bc[:],
                            in_=bass.AP(tensor=dsi_d.ap().tensor, offset=h * NQ,
                                        ap=[[0, 64], [1, NQ]]))
                    return f

                def mul_chunk(e, half):
                    def f():
                        h = 2 * hp + e
                        sl = slice(half * 512, (half + 1) * 512)
                        if e == 0:
                            nc.vector.tensor_mul(out=pairT[hp][0:64, sl],
                                                 in0=numT[h][0:64, sl],
                                                 in1=dbs[e][:, sl])
                        else:
                            nc.vector.tensor_mul(out=numT[h][0:64, sl],
                                                 in0=numT[h][0:64, sl],
                                                 in1=dbs[e][:, sl])
                    return f

                def stitch():
                    nc.sync.dma_start(out=pairT[hp][64:128, :],
                                      in_=numT[2 * hp + 1][0:64, :])

                return [p0, None, None, None, p1, None, load_bc(0),
                        load_bc(1), None, None, mul_chunk(0, 0),
                        mul_chunk(0, 1), mul_chunk(1, 0), mul_chunk(1, 1),
                        stitch]

            # ---- build the B-work queue (popped into C's iterations) -------
            for t in range(12):
                ln_stats(t)
                if t % 4 == 3:
                    roundtrip(t // 4)

            # Prologue emission (before C iter0): everything iter0 needs.
            rstdT_build(0)[0]()
            k_chunk(0, 0)[0]()
            q_chunk(0, 0)[0]()
            v_tile(0)[0]()
            v_tile(1)[0]()

            # Remaining B-work, popped 2/iter inside C. Order respects
            # both data deadlines and PE program-order (producer closures
            # must be emitted before their consumers hit the PE queue).
            workq = [lambda t=t: ln_stats(t) for t in range(12, NT)]
            workq += [lambda: roundtrip(3)]
            workq += rstdT_build(1)
            workq += k_chunk(0, 1)
            workq += v_tile(2)
            workq += v_tile(3)
            workq += rstdT_build(2)
            workq += k_chunk(0, 2)
            workq += v_tile(4)
            workq += v_tile(5)
            workq += rstdT_build(3)
            workq += k_chunk(0, 3)
            for t in range(6, 9):
                workq += v_tile(t)
            workq += q_chunk(0, 1)
            for t in range(9, NT):
                workq += v_tile(t)
            # production for later passes: pass p=2..7 uses K[p//2],
            # Q[p//2, p%2]; all of it drains well before pass 2 starts.
            for hp in range(1, 4):
                for c in range(4):
                    workq += k_chunk(hp, c)
                workq += q_chunk(hp, 0)
                workq += q_chunk(hp, 1)

            # ---- Phase C: 8 passes x 16 key tiles --------------------------
            avq = []   # (closure) AV work trailing the exp stream

            def av_mms(p, jt, pbt, av0, av1):
                def f():
                    hp = p // 2
                    h0, h1 = 2 * hp, 2 * hp + 1
                    nc.tensor.matmul(av0[:], Vau[jt][:, h0, 0:65],
                                     pbt[:, 0:512],
                                     start=(jt == 0), stop=(jt == NT - 1))
                    nc.tensor.matmul(av1[:], Vau[jt][:, h1, 0:65],
                                     pbt[:, 512:1024],
                                     start=(jt == 0), stop=(jt == NT - 1))
                return f

            for p in range(8):
                hp, ic = p // 2, p % 2
                av0 = psC.tile([65, 512], F32, tag="av0", bufs=1,
                               name=f"av0_{p}")
                av1 = psC.tile([65, 512], F32, tag="av1", bufs=1,
                               name=f"av1_{p}")
                i5 = ic * 512
                for jt in range(NT):
                    sp = psC.tile([128, 1024], F32, tag="sp", bufs=2)
                    nc.tensor.matmul(
                        sp[:, 0:512],
                        KTp[hp][0:64, jt * 128:(jt + 1) * 128],
                        QTp[hp][0:64, i5:i5 + 512],
                        start=True, stop=True, tile_position=(0, 0))
                    nc.tensor.matmul(
                        sp[:, 512:1024],
                        KTp[hp][64:128, jt * 128:(jt + 1) * 128],
                        QTp[hp][64:128, i5:i5 + 512],
                        start=True, stop=True, tile_position=(64, 0))
                    eb = W.tile([128, 1024], BF16, tag="eb", bufs=3)
                    nc.scalar.activation(out=eb[:], in_=sp[:], func=AF.Exp)
                    pbt = W.tile([128, 1024], BF16, tag="pb", bufs=LAG + 3)
                    nc.vector.tensor_mul(out=pbt[:, 0:512], in0=eb[:, 0:512],
                                         in1=expA[jt][:, i5:i5 + 512])
                    nc.vector.tensor_mul(out=pbt[:, 512:1024],
                                         in0=eb[:, 512:1024],
                                         in1=expA[jt][:, i5:i5 + 512])
                    avq.append(av_mms(p, jt, pbt, av0, av1))
                    # drain trailing B-work (2/iter keeps producers ahead
                    # of their PE-queue consumers), then lagged AV work
                    # (also at most 2/iter so den roundtrips stay spread).
                    for _ in range(2):
                        if workq:
                            workq.pop(0)()
                    for _ in range(2):
                        if len(avq) > LAG:
                            fn = avq.pop(0)
                            if fn is not None:
                                fn()
                # end of pass: trailing AV for this pass still in avq; queue
                # the evacuation work behind them.
                def pass_tail(p=p, av0=av0, av1=av1):
                    hp, ic = p // 2, p % 2
                    h0, h1 = 2 * hp, 2 * hp + 1
                    sl = slice(ic * 512, (ic + 1) * 512)
                    def f():
                        nc.vector.tensor_copy(out=numT[h0][:, sl], in_=av0[:])
                        nc.vector.tensor_copy(out=numT[h1][:, sl], in_=av1[:])
                        if ic == 1:
                            for e in range(2):
                                h = 2 * hp + e
                                nc.sync.dma_start(out=dsb_d[h, :],
                                                  in_=numT[h][64:65, :])
                    return f
                avq.append(pass_tail())
                if p % 2 == 1:
                    avq.extend(den_pieces(p // 2))

            # flush remaining trailing work, keeping the PE busy through the
            # last denominator roundtrip (dummy matmuls bridge HAM warmth)
            tail = [fn for fn in avq if fn is not None]
            for i, fn in enumerate(tail):
                fn()
                if i % 2 == 1:
                    dmy = psB.tile([128, 512], F32, tag="bps", bufs=2,
                                   name=f"dmy{i}")
                    nc.tensor.matmul(dmy[:], wqkv_sb[0][:, 0:128],
                                     xt_sb[0][:, 0:512], start=True, stop=True)

        # ---- Phase D: output projection (head pairs, K=128) ----------------
        with tc.tile_pool(name="psD", bufs=1, space="PSUM") as psD:
            yps = [psD.tile([128, 512], F32, tag=f"yp{isl}", name=f"yp{isl}")
                   for isl in range(8)]
            for hp in range(4):
                for isl in range(8):
                    nc.tensor.matmul(yps[isl][:],
                                     pairT[hp][:, isl * 128:(isl + 1) * 128],
                                     woutP[hp][:],
                                     start=(hp == 0), stop=(hp == 3))
            for isl in range(8):
                ysb = W.tile([128, 512], F32, tag="ysb", bufs=4)
                if has_b:
                    nc.vector.tensor_add(out=ysb[:], in0=yps[isl][:],
                                         in1=bout_bc[:])
                elif isl % 2 == 0:
                    nc.vector.tensor_copy(out=ysb[:], in_=yps[isl][:])
                else:
                    nc.scalar.copy(out=ysb[:], in_=yps[isl][:])
                nc.sync.dma_start(out=y_d[isl * 128:(isl + 1) * 128, :],
                                  in_=ysb[:])
    if split_waits:
        _split_waits(nc)
    return nc


_NC_CACHE = {}


def _get_nc(has_c, has_b):
    key = (has_c, has_b)
    if key not in _NC_CACHE:
        _NC_CACHE[key] = build(has_c, has_b)
    return _NC_CACHE[key]


LAST_EXEC_TIME_NS = None


def kernel(x, gamma, beta, Wqkv, Wout, bout, rel_table, temporal_mask,
           trace=True):
    global LAST_EXEC_TIME_NS
    x = np.asarray(x, np.float32)
    gamma = np.asarray(gamma, np.float32)
    beta = np.asarray(beta, np.float32)
    Wqkv = np.asarray(Wqkv, np.float32)
    Wout = np.asarray(Wout, np.float32)
    bout = np.asarray(bout, np.float32)
    rel_table = np.asarray(rel_table, np.float32)
    temporal_mask = np.asarray(temporal_mask)

    scale = DH ** -0.5
    w_eff = (Wqkv * gamma[:, None]).copy()
    w_eff[:, :D] *= scale
    cqkv = (beta @ Wqkv).astype(np.float32)
    cqkv[:D] *= scale
    wqkv_bf = w_eff.astype(ml_dtypes.bfloat16)
    # rank-1 mean correction: -g rows (so the matmul accumulates -mu*g)
    grow_bf = (-w_eff.sum(axis=0)).astype(ml_dtypes.bfloat16)
    wout_bf = Wout.astype(ml_dtypes.bfloat16)
    mask01 = (temporal_mask != 0)

    idx = np.arange(N)
    # expbias[i, j] = exp(rel_table[clip(i - j)]) with i=query, j=key
    expbias = np.exp(rel_table[
        np.clip(idx[:, None] - idx[None, :], -(MAXREL - 1), MAXREL - 1)
        + MAXREL - 1]).astype(np.float32)

    keyperm_half = [
        np.concatenate([np.arange(i0, i0 + NQ),
                        np.arange(NQ - i0, NQ - i0 + NQ)])
        for i0 in (0, NQ)
    ]
    # expA[j_perm, i_local] = exp(bias(query i, key j)) * mask(query i, key j)
    expa_half = []
    for half in range(2):
        kp = keyperm_half[half]
        qs = np.arange(half * NQ, (half + 1) * NQ)
        a = (expbias[np.ix_(qs, kp)] * mask01[np.ix_(qs, kp)]).T
        expa_half.append(np.ascontiguousarray(a).astype(ml_dtypes.bfloat16))

    in_maps = []
    for c in range(NCORES):
        b, half = c // 2, c % 2
        xp = np.ascontiguousarray(x[b][keyperm_half[half]])
        xtp = np.ascontiguousarray(xp.T).astype(ml_dtypes.bfloat16)
        in_maps.append({
            "x": xp,
            "xt": xtp,
            "wqkv": wqkv_bf,
            "grow": grow_bf,
            "cqkv": cqkv,
            "wout": wout_bf,
            "bout": bout,
            "expa": expa_half[half],
        })

    nc = _get_nc(bool(np.any(cqkv != 0.0)), bool(np.any(bout != 0.0)))
    res = run_bass_kernel_spmd(nc, in_maps, core_ids=list(range(NCORES)),
                               trace=trace)
    LAST_EXEC_TIME_NS = res.exec_time_ns

    out = np.empty((B, N, D), np.float32)
    for c in range(NCORES):
        b, half = c // 2, c % 2
        out[b, half * NQ:(half + 1) * NQ] = res.results[c]["y"]
    return out
